# revision 1
# baseline (speedup 1.0000x reference)
"""Block-quantize kernel for Trainium2 (8 NeuronCores, data-parallel).

Reference semantics (fp32, wl=8, ebit=8):
    m  = max(max|x|, 1e-10)                      # global over all elements
    e  = clip(floor(log2(m)), -128, 127)
    y  = clip(round_half_even(x * 2^(6-e)), -128, 127) * 2^(e-6)

Implementation:
  - x (16, 2048, 4096) f32 is sharded on the batch dim: 2 batches per core,
    viewed per-core as (2048, 8192) so each [128, 8192] tile is one
    contiguous 4 MiB DMA.
  - Pass 1 streams the shard computing abs-max (DVE tensor_reduce with
    apply_absolute_value), reduces across partitions (GpSimd
    partition_all_reduce), then a 4-byte AllReduce(max) across the 8 cores.
  - e and the two power-of-two scales are derived with exact int32 bit
    arithmetic on the fp32 representation (all values are multiples of 2^23
    with small multipliers, so the DVE's internal fp32 math is exact):
        p  = bits(m) & 0x7F800000            # bits of 2^e
        s2 = bits^-1(p - (6<<23))            # 2^(e-6)
        s1 = bits^-1(((254<<23) - p) + (6<<23))   # 2^(6-e)
  - Pass 2 streams the shard again:
        r  = x*s1 + C        # C = 1.5*2^23; fp32 RNE add == round-half-even
        u  = min(r, C+127) ; max(u, C-128)        # clip in shifted domain
        y  = (u - C) * s2                         # both steps exact in fp32
    Every elementwise op is a dual-op DVE tensor_scalar (2x perf mode).
"""
import sys

if "/opt/trn_rl_repo" not in sys.path:
    sys.path.insert(0, "/opt/trn_rl_repo")

import numpy as np

N_CORES = 8
B, S, D = 16, 2048, 4096          # full input shape
PB = B // N_CORES                  # batches per core
P = 128                            # SBUF partitions
F = 8192                           # tile free dim  -> [128, 8192] = 4 MiB
ROWS = PB * S * D // F             # 2048 per-core rows of 8192
T = ROWS // P                      # 16 tiles per core
C_MAGIC = 12582912.0               # 1.5 * 2^23, round-to-nearest-even magic

_CACHE = {}


def _build():
    import concourse.mybir as mybir
    from concourse import bacc, bass_isa, tile

    DT = mybir.dt.float32
    DI = mybir.dt.int32
    A = mybir.AluOpType

    nc = bacc.Bacc("TRN2", target_bir_lowering=False, debug=False,
                   num_devices=N_CORES)
    x = nc.dram_tensor("x", [ROWS, F], DT, kind="ExternalInput")
    y = nc.dram_tensor("y", [ROWS, F], DT, kind="ExternalOutput")

    with tile.TileContext(nc) as tc:
        with tc.tile_pool(name="data", bufs=5) as data, \
             tc.tile_pool(name="small", bufs=1) as small, \
             tc.tile_pool(name="dram", bufs=1, space="DRAM") as dram:

            # ---------------- pass 1: local abs-max ----------------
            stats = small.tile([P, T], DT)
            for i in range(T):
                t = data.tile([P, F], DT, tag="blk")
                nc.sync.dma_start(out=t[:], in_=x[i * P:(i + 1) * P, :])
                nc.vector.tensor_reduce(out=stats[:, i:i + 1], in_=t[:],
                                        axis=mybir.AxisListType.X,
                                        op=A.max, apply_absolute_value=True)
            lmax = small.tile([P, 1], DT)
            nc.vector.tensor_reduce(out=lmax[:], in_=stats[:],
                                    axis=mybir.AxisListType.X, op=A.max)
            amax = small.tile([P, 1], DT)
            nc.gpsimd.partition_all_reduce(amax[:], lmax[:], channels=P,
                                           reduce_op=bass_isa.ReduceOp.max)
            # zeros map to 1e-10 in the reference, so m >= 1e-10
            nc.vector.tensor_scalar(out=amax[:], in0=amax[:], scalar1=1e-10,
                                    scalar2=None, op0=A.max)

            # -------- all-reduce(max) of one scalar across 8 cores --------
            cc_in = dram.tile([1, 1], DT)
            cc_out = dram.tile([1, 1], DT)
            nc.sync.dma_start(out=cc_in[:], in_=amax[0:1, 0:1])
            nc.gpsimd.collective_compute(
                "AllReduce", A.max,
                replica_groups=[list(range(N_CORES))],
                ins=[cc_in[:]], outs=[cc_out[:]],
            )
            gm1 = small.tile([1, 1], DT)
            nc.sync.dma_start(out=gm1[:], in_=cc_out[:])
            gmax = small.tile([P, 1], DT)
            nc.gpsimd.partition_broadcast(gmax[:], gm1[:])

            # ---------------- scales via exact bit arithmetic ----------------
            bits = gmax[:].bitcast(DI)
            p_i = small.tile([P, 1], DI)
            nc.vector.tensor_scalar(out=p_i[:], in0=bits, scalar1=0x7F800000,
                                    scalar2=None, op0=A.bitwise_and)
            s2i = small.tile([P, 1], DI)
            nc.vector.tensor_scalar(out=s2i[:], in0=p_i[:], scalar1=6 << 23,
                                    scalar2=None, op0=A.subtract)
            s1i = small.tile([P, 1], DI)
            nc.vector.tensor_scalar(out=s1i[:], in0=p_i[:], scalar1=254 << 23,
                                    scalar2=-1.0, op0=A.subtract, op1=A.mult)
            nc.vector.tensor_scalar(out=s1i[:], in0=s1i[:], scalar1=6 << 23,
                                    scalar2=None, op0=A.add)
            s1 = s1i[:].bitcast(DT)
            s2 = s2i[:].bitcast(DT)

            # ---------------- pass 2: quantize ----------------
            for i in range(T):
                t = data.tile([P, F], DT, tag="blk")
                nc.sync.dma_start(out=t[:], in_=x[i * P:(i + 1) * P, :])
                nc.vector.tensor_scalar(out=t[:], in0=t[:], scalar1=s1,
                                        scalar2=C_MAGIC,
                                        op0=A.mult, op1=A.add)
                nc.vector.tensor_scalar(out=t[:], in0=t[:],
                                        scalar1=C_MAGIC + 127.0,
                                        scalar2=C_MAGIC - 128.0,
                                        op0=A.min, op1=A.max)
                nc.vector.tensor_scalar(out=t[:], in0=t[:], scalar1=-C_MAGIC,
                                        scalar2=s2, op0=A.add, op1=A.mult)
                nc.sync.dma_start(out=y[i * P:(i + 1) * P, :], in_=t[:])

    nc.compile()
    return nc


def _get_nc():
    if "nc" not in _CACHE:
        _CACHE["nc"] = _build()
    return _CACHE["nc"]


def kernel(x: np.ndarray) -> np.ndarray:
    from concourse.bass_utils import run_bass_kernel_spmd

    x = np.ascontiguousarray(x, dtype=np.float32)
    assert x.shape == (B, S, D), x.shape
    nc = _get_nc()
    in_maps = [
        {"x": x[c * PB:(c + 1) * PB].reshape(ROWS, F)} for c in range(N_CORES)
    ]
    res = run_bass_kernel_spmd(nc, in_maps, core_ids=list(range(N_CORES)))
    out = np.empty((B, S, D), dtype=np.float32)
    for c in range(N_CORES):
        out[c * PB:(c + 1) * PB] = res.results[c]["y"].reshape(PB, S, D)
    return out
